# revision 1
# baseline (speedup 1.0000x reference)
"""DeepWalk hierarchical-softmax loss kernel for Trainium2 (8 NeuronCores).

Computation (per the nn.Module reference):
    ctx, leaf = edges[:, 0], edges[:, 1]
    nodes = path_nodes[leaf]            # [B, L]
    signs = path_signs[leaf]            # [B, L]
    mask  = path_mask[leaf]             # [B, L]
    x     = einsum("bd,bld->bl", Z[ctx], Z[nodes])
    loss  = -sum(where(mask, log_sigmoid(signs * x), 0))
          = +sum(where(mask, softplus(-signs * x), 0))

Sharding: data-parallel over the edge batch; 8 cores x 4096 edges.
Z and the path tables are replicated to every core. Each core emits
[128, 1] per-partition partial sums; the host adds them up (that's the
unshard step for a loss output).

Device-side algorithm per core (P=128 partitions, T=32 edge tiles):
    - edges arrive pre-transposed as ctx/leaf int32 [P, T] (host layout prep)
    - indirect-DMA gather path rows:   nodes/signs [P, T*L] i32, mask u8
    - indirect-DMA gather Z[ctx]:      zv [P, T*D] f32
    - per edge-tile t: indirect-DMA gather Z rows for nodes[:, t*L:(t+1)*L]
      into zp [P, L*D]; DVE multiply by zv broadcast over L; DVE segmented
      reduce over D -> x[:, t*L:(t+1)*L]
    - epilogue: h = x*(signs*mask) + BIG*(1-mask);  ACT softplus(-h) with
      accum_out -> [P, 1] partial sums  (masked slots give softplus(-BIG)=0)
"""

import dataclasses
import os
import tempfile

# The neuronx-cc on-disk compile cache keys on the HLO graph hash, which does
# NOT include the bass_exec backend_config (the embedded BIR). Two different
# kernel builds with the same I/O signature therefore collide, and a stale
# NEFF from an earlier build would silently run instead of this one. Use a
# fresh per-process cache dir, set before libneuronxla reads the env.
os.environ.setdefault(
    "NEURON_COMPILE_CACHE_URL", tempfile.mkdtemp(prefix="neuron_cc_cache_")
)

import numpy as np

import concourse.bacc as bacc
import concourse.bass as bass
import concourse.mybir as mybir
import concourse.tile as tile
from concourse.bass import IndirectOffsetOnAxis
from concourse.bass_utils import run_bass_kernel_spmd

P = 128


@dataclasses.dataclass(frozen=True)
class DeepWalkCfg:
    n_leaves: int = 500_000       # path-table rows
    n_nodes: int = 999_999        # Z rows
    depth: int = 20               # L
    dim: int = 128                # D
    edges_per_core: int = 4096    # B / n_cores
    n_cores: int = 8
    big: float = 50.0             # masked slots: softplus(-big) == 0 in f32

    @property
    def t_tiles(self) -> int:
        assert self.edges_per_core % P == 0
        return self.edges_per_core // P


def build_deepwalk(tc: tile.TileContext, outs, ins, cfg: DeepWalkCfg, dbg=None):
    nc = tc.nc
    (out_d,) = outs
    ctx_d, leaf_d, pnodes_d, psigns_d, pmask_d, z_d = ins
    T, L, D = cfg.t_tiles, cfg.depth, cfg.dim
    f32 = mybir.dt.float32

    with (
        tc.tile_pool(name="const", bufs=1) as cpool,
        tc.tile_pool(name="zp", bufs=4) as zp_pool,
        tc.tile_pool(name="prod", bufs=3) as prod_pool,
    ):
        ctx_s = cpool.tile([P, T], mybir.dt.int32)
        leaf_s = cpool.tile([P, T], mybir.dt.int32)
        nc.sync.dma_start(out=ctx_s[:], in_=ctx_d[:, :])
        nc.sync.dma_start(out=leaf_s[:], in_=leaf_d[:, :])

        # NOTE (HW-probed): indirect DMA pairs offsets with dest chunks
        # correctly ONLY for [P, 1]-shaped offset APs — one gathered row per
        # partition per instruction. Multi-column offset APs scramble
        # (walrus reads just two offsets per partition and auto-increments).
        nodes_all = cpool.tile([P, T * L], mybir.dt.int32)
        signs_all = cpool.tile([P, T * L], mybir.dt.int32)
        mask_all = cpool.tile([P, T * L], mybir.dt.uint8)
        for t in range(T):
            for dst, src in ((nodes_all, pnodes_d), (signs_all, psigns_d), (mask_all, pmask_d)):
                nc.gpsimd.indirect_dma_start(
                    out=dst[:, t * L : (t + 1) * L],
                    out_offset=None,
                    in_=src[:, :],
                    in_offset=IndirectOffsetOnAxis(ap=leaf_s[:, t : t + 1], axis=0),
                )

        zv_all = cpool.tile([P, T * D], f32)
        for t in range(T):
            nc.gpsimd.indirect_dma_start(
                out=zv_all[:, t * D : (t + 1) * D],
                out_offset=None,
                in_=z_d[:, :],
                in_offset=IndirectOffsetOnAxis(ap=ctx_s[:, t : t + 1], axis=0),
            )

        x_all = cpool.tile([P, T * L], f32)
        for t in range(T):
            zp_t = zp_pool.tile([P, L * D], f32)
            for l in range(L):
                nc.gpsimd.indirect_dma_start(
                    out=zp_t[:, l * D : (l + 1) * D],
                    out_offset=None,
                    in_=z_d[:, :],
                    in_offset=IndirectOffsetOnAxis(
                        ap=nodes_all[:, t * L + l : t * L + l + 1], axis=0
                    ),
                )
            prod_t = prod_pool.tile([P, L * D], f32)
            zv_b = zv_all[:, t * D : (t + 1) * D].unsqueeze(1).to_broadcast([P, L, D])
            nc.vector.tensor_tensor(
                out=prod_t[:].rearrange("p (l d) -> p l d", d=D),
                in0=zp_t[:].rearrange("p (l d) -> p l d", d=D),
                in1=zv_b,
                op=mybir.AluOpType.mult,
            )
            nc.vector.tensor_reduce(
                out=x_all[:, t * L : (t + 1) * L],
                in_=prod_t[:].rearrange("p (l d) -> p l d", d=D),
                axis=mybir.AxisListType.X,
                op=mybir.AluOpType.add,
            )

        # epilogue: per-element loss = mask * softplus(-w), w = x*sign.
        # Exact, range-safe split (the HW Ln table is only valid on
        # ~[3e-20, 3e19]): softplus(-w) = relu(-w) + ln(1 + exp(-|w|)),
        # where the Ln argument always lies in [1, 2].
        # NOTE: plain tensor_scalar hangs this runtime (HW-probed); use the
        # scalar_tensor_tensor form with op1=bypass instead.
        s_f = cpool.tile([P, T * L], f32)
        m_f = cpool.tile([P, T * L], f32)
        nc.vector.tensor_copy(out=s_f[:], in_=signs_all[:])
        nc.vector.tensor_copy(out=m_f[:], in_=mask_all[:])
        w = cpool.tile([P, T * L], f32)
        nc.vector.tensor_tensor(out=w[:], in0=x_all[:], in1=s_f[:], op=mybir.AluOpType.mult)
        aw = cpool.tile([P, T * L], f32)
        nc.scalar.activation(out=aw[:], in_=w[:], func=mybir.ActivationFunctionType.Abs)
        e2 = cpool.tile([P, T * L], f32)
        nc.scalar.activation(
            out=e2[:], in_=aw[:], func=mybir.ActivationFunctionType.Exp, scale=-1.0
        )
        p1 = cpool.tile([P, T * L], f32)
        nc.vector.scalar_tensor_tensor(
            out=p1[:], in0=e2[:], scalar=1.0, in1=e2[:],
            op0=mybir.AluOpType.add, op1=mybir.AluOpType.bypass,
        )
        lnp = cpool.tile([P, T * L], f32)
        nc.scalar.activation(
            out=lnp[:], in_=p1[:], func=mybir.ActivationFunctionType.Ln
        )
        r = cpool.tile([P, T * L], f32)
        nc.scalar.activation(
            out=r[:], in_=w[:], func=mybir.ActivationFunctionType.Relu, scale=-1.0
        )
        sp = cpool.tile([P, T * L], f32)
        nc.vector.tensor_tensor(out=sp[:], in0=r[:], in1=lnp[:], op=mybir.AluOpType.add)
        junk = cpool.tile([P, T * L], f32)
        acc = cpool.tile([P, 1], f32)
        nc.vector.scalar_tensor_tensor(
            out=junk[:], in0=sp[:], scalar=0.0, in1=m_f[:],
            op0=mybir.AluOpType.add, op1=mybir.AluOpType.mult, accum_out=acc[:],
        )
        nc.sync.dma_start(out=out_d[:, :], in_=acc[:])
        if dbg is not None:
            for name, t in (("mask", mask_all), ("signs", signs_all),
                            ("nodes", nodes_all), ("x", x_all), ("sp", sp)):
                if name in dbg:
                    nc.sync.dma_start(out=dbg[name][:, :], in_=t[:])


def build_module(cfg: DeepWalkCfg) -> bacc.Bacc:
    nc = bacc.Bacc("TRN2", target_bir_lowering=False, debug=False, num_devices=cfg.n_cores)
    T, L, D = cfg.t_tiles, cfg.depth, cfg.dim
    i32, u8, f32 = mybir.dt.int32, mybir.dt.uint8, mybir.dt.float32
    ins = [
        nc.dram_tensor("ctx", [P, T], i32, kind="ExternalInput").ap(),
        nc.dram_tensor("leaf", [P, T], i32, kind="ExternalInput").ap(),
        nc.dram_tensor("pnodes", [cfg.n_leaves, L], i32, kind="ExternalInput").ap(),
        nc.dram_tensor("psigns", [cfg.n_leaves, L], i32, kind="ExternalInput").ap(),
        nc.dram_tensor("pmask", [cfg.n_leaves, L], u8, kind="ExternalInput").ap(),
        nc.dram_tensor("Z", [cfg.n_nodes, D], f32, kind="ExternalInput").ap(),
    ]
    outs = [nc.dram_tensor("out", [P, 1], f32, kind="ExternalOutput").ap()]
    with tile.TileContext(nc) as tc:
        build_deepwalk(tc, outs, ins, cfg)
    nc.compile()
    return nc


_NC_CACHE: dict = {}


def _get_module(cfg: DeepWalkCfg) -> bacc.Bacc:
    if cfg not in _NC_CACHE:
        _NC_CACHE[cfg] = build_module(cfg)
    return _NC_CACHE[cfg]


def shard_inputs(edges, path_nodes, path_signs, path_mask, Z, cfg: DeepWalkCfg):
    """Host-side shard + layout prep. Returns in_maps for run_bass_kernel_spmd."""
    edges = np.asarray(edges)
    pnodes = np.ascontiguousarray(np.asarray(path_nodes, dtype=np.int32))
    psigns = np.ascontiguousarray(np.asarray(path_signs, dtype=np.int32))
    pmask = np.ascontiguousarray(np.asarray(path_mask)).view(np.uint8)
    z = np.ascontiguousarray(np.asarray(Z, dtype=np.float32))
    epc, T = cfg.edges_per_core, cfg.t_tiles
    in_maps = []
    for c in range(cfg.n_cores):
        sh = edges[c * epc : (c + 1) * epc]  # [epc, 2]
        # [T*P, 2] -> per-tile partition-major [P, T]
        ctx = np.ascontiguousarray(sh[:, 0].reshape(T, P).T).astype(np.int32)
        leaf = np.ascontiguousarray(sh[:, 1].reshape(T, P).T).astype(np.int32)
        in_maps.append(
            {"ctx": ctx, "leaf": leaf, "pnodes": pnodes, "psigns": psigns,
             "pmask": pmask, "Z": z}
        )
    return in_maps


def kernel(edges, path_nodes, path_signs, path_mask, Z, _results_out=None, **run_kwargs) -> np.ndarray:
    cfg = DeepWalkCfg()
    b = np.asarray(edges).shape[0]
    assert b == cfg.edges_per_core * cfg.n_cores, (b, cfg)
    nc = _get_module(cfg)
    in_maps = shard_inputs(edges, path_nodes, path_signs, path_mask, Z, cfg)
    res = run_bass_kernel_spmd(nc, in_maps, core_ids=list(range(cfg.n_cores)), **run_kwargs)
    if _results_out is not None:
        _results_out["results"] = res
    # device emits per-partition sums of softplus(-h); loss = sum(...)
    total = np.float64(0.0)
    for r in res.results:
        total += np.asarray(r["out"], dtype=np.float64).sum()
    return np.float32(total)

